# revision 1
# baseline (speedup 1.0000x reference)
"""Multi-head attention (12 heads, RoPE, causal SDPA) for Trainium2, 8 cores.

Sharding: batch (2) x head-group (4 groups of 3 heads). Each core computes,
for its (batch b, head-group hg): QKV projection for its 3 heads, RoPE,
causal attention, and a partial out-projection [T, C] restricted to its
heads' rows of w_out. The host sums the 4 head-group partials per batch.

Device-side layouts (T = 2048, C = 768, D = 64 per head):
  xT   [768, 2048]  x[b] transposed (c on partitions)
  wA   [768, 640]   packed lhsT weights: cols 0:128 [q0|q1], 128:256 [k0|k1],
                    256:320 q2, 320:384 k2, 384:576 w_v (3 heads), 64 zero pad
  wo   [64, 2304]   w_out rows for this head-group: 3 x [64 d, 768 c]
  cosT/sinT [128, 2048]  RoPE tables transposed, stacked twice (64 d x 2)
  rT   [128, 128]   rotate_half as matmul lhsT: rot(q)T_chunk = rT.T @ qT_chunk
  tri  [128, 128]   tri[kr, qc] = 1 if qc >= kr (causal keep-mask, S^T layout)

Attention is computed transposed (S^T[k, q] = K Q^T blocks) so that softmax
P^T lands in [k, q] layout, which feeds P@V directly with v in natural [t, d]
layout (no transposes). Softmax has no max-subtraction (scores are O(1) by
construction) and the denominator comes from an all-ones column appended to
the stationary v operand. Normalization is applied in the [d, q] layout via a
K=1 outer-product broadcast of 1/denominator. Matmuls use float32r (~13
mantissa bits, 4x faster than fp32 on the PE).
"""
import numpy as np

B, T, C, H, D = 2, 2048, 768, 12, 64
HPG = 3                    # heads per group
NG = B * (H // HPG)        # 8 cores
ROPE_BASE = 10000.0
TQ = T // 128              # 16 t-tiles
NCC = C // 128             # 6 contraction chunks
GW = 1024                  # attention q-group width
NGRP = T // GW             # 2 q-groups

_CACHE = {}


def _build_nc(reps=1):
    from concourse import bacc, tile, mybir

    f32 = mybir.dt.float32
    f32r = mybir.dt.float32r
    Exp = mybir.ActivationFunctionType.Exp
    mult = mybir.AluOpType.mult
    add = mybir.AluOpType.add

    nc = bacc.Bacc("TRN2", target_bir_lowering=False, debug=False,
                   num_devices=NG)

    xT_d = nc.dram_tensor("xT", [C, T], f32r, kind="ExternalInput").ap()
    wA_d = nc.dram_tensor("wA", [C, 640], f32r, kind="ExternalInput").ap()
    woA_d = nc.dram_tensor("woA", [2 * D, C], f32r, kind="ExternalInput").ap()
    woB_d = nc.dram_tensor("woB", [D, C], f32r, kind="ExternalInput").ap()
    cosT_d = nc.dram_tensor("cosT", [128, T], f32, kind="ExternalInput").ap()
    sinT_d = nc.dram_tensor("sinT", [128, T], f32, kind="ExternalInput").ap()
    rT_d = nc.dram_tensor("rT", [128, 128], f32r, kind="ExternalInput").ap()
    tri_d = nc.dram_tensor("tri", [128, 128], f32, kind="ExternalInput").ap()
    out_d = nc.dram_tensor("out", [T, C], f32, kind="ExternalOutput").ap()

    with tile.TileContext(nc) as tc:
      for rep in range(reps):
        with tc.tile_pool(name=f"persist{rep}", bufs=1) as pp:
                dmaq = [nc.sync, nc.scalar, nc.gpsimd]

                # ---- persistent constants ----
                woA = pp.tile([2 * D, C], f32r, tag="woA")
                dmaq[1].dma_start(woA[:], woA_d[:])
                woB = pp.tile([D, C], f32r, tag="woB")
                dmaq[1].dma_start(woB[:], woB_d[:])
                tri = pp.tile([128, 128], f32, tag="tri")
                dmaq[2].dma_start(tri[:], tri_d[:])
                onesf = pp.tile([1, D], f32, tag="onesf")
                nc.vector.memset(onesf[:], 1.0)
                ones = pp.tile([1, D], f32r, tag="ones")
                nc.scalar.copy(ones[:], onesf[:])

                # persistent intermediates: [q0|q1], [k0|k1], [q2], [k2]
                # (projection computes [q2|k2] packed; RoPE splits into two
                # 64-row tiles via cross-partition DVE writes)
                qk_rows = [128, 128, 64, 64]
                qkT = [pp.tile([qk_rows[m], T], f32r, tag=f"qkT{m}",
                               name=f"qkT{m}") for m in range(4)]
                v_sb = pp.tile([128, TQ, HPG, 65], f32r, tag="v_sb")
                onesw = pp.tile([128, TQ * HPG], f32, tag="onesw")
                nc.vector.memset(onesw[:], 1.0)
                nc.scalar.copy(
                    v_sb[:, :, :, 64:65],
                    onesw[:].rearrange("p (a b) -> p a b", b=HPG).rearrange(
                        "p a b -> p a b ()"))
                attnT_A = pp.tile([2 * D, T], f32r, tag="attnTA")
                attnT_B = pp.tile([D, T], f32r, tag="attnTB")
                attn_dst = [(attnT_A, 0), (attnT_A, D), (attnT_B, 0)]

                # ================= QKV phase (scoped pools) =================
                qkv_pool = tc.tile_pool(name=f"qkv{rep}", bufs=1)
                qp = qkv_pool.__enter__()
                qkv_ps_pool = tc.tile_pool(name=f"qkvps{rep}", bufs=8, space="PSUM")
                qps = qkv_ps_pool.__enter__()

                xT = [qp.tile([128, T], f32r, tag=f"xT{c}", name=f"xT{c}")
                      for c in range(NCC)]
                wA = [qp.tile([128, 640], f32r, tag=f"wA{c}", name=f"wA{c}")
                      for c in range(NCC)]
                # weights first (small), then xT column-major in [128, 512]
                # pieces so the first projection chunk's deps arrive in ~2us
                for c in range(NCC):
                    dmaq[c % 3].dma_start(
                        wA[c][:], wA_d[128 * c:128 * (c + 1), :])
                qi = 0
                for n in range(4):
                    for c in range(NCC):
                        nsl = slice(512 * n, 512 * (n + 1))
                        dmaq[qi % 3].dma_start(
                            xT[c][:, nsl], xT_d[128 * c:128 * (c + 1), nsl])
                        qi += 1
                cosT = qp.tile([128, T], f32, tag="cosT")
                sinT = qp.tile([128, T], f32, tag="sinT")
                dmaq[2].dma_start(cosT[:], cosT_d[:])
                dmaq[0].dma_start(sinT[:], sinT_d[:])
                rT = qp.tile([128, 128], f32r, tag="rT")
                dmaq[1].dma_start(rT[:], rT_d[:])

                # q/k projection + RoPE; rot matmuls lag the raw projections
                # by two chunks so PE never stalls on the ACT psum->sbuf copy
                qk_cols = [(0, 128), (128, 256), (256, 384)]
                chunks = [(m, n) for n in range(4) for m in range(3)]
                raws = {}

                def emit_raw(i):
                    m, n = chunks[i]
                    c0, c1 = qk_cols[m]
                    rows = 128
                    tsl = slice(512 * n, 512 * (n + 1))
                    praw = qps.tile([128, 512], f32, tag="ps", name=f"praw{i}")
                    for c in range(NCC):
                        nc.tensor.matmul(
                            praw[0:rows, :], wA[c][:, c0:c1], xT[c][:, tsl],
                            start=(c == 0), stop=(c == NCC - 1))
                    raw = qp.tile([128, 512], f32r, tag="raw", bufs=5,
                                  name=f"raw{i}")
                    nc.scalar.copy(raw[0:rows, :], praw[0:rows, :])
                    raws[i] = raw

                def emit_rope(i):
                    m, n = chunks[i]
                    tsl = slice(512 * n, 512 * (n + 1))
                    raw = raws.pop(i)
                    prot = qps.tile([128, 512], f32, tag="ps", name=f"prot{i}")
                    nc.tensor.matmul(prot[:], rT[:], raw[:], start=True,
                                     stop=True)
                    t1 = qp.tile([128, 512], f32, tag="t1", bufs=3,
                                 name=f"t1_{i}")
                    nc.gpsimd.tensor_tensor(t1[:], raw[:], cosT[:, tsl], mult)
                    t2 = qp.tile([128, 512], f32, tag="t2", bufs=3,
                                 name=f"t2_{i}")
                    nc.vector.tensor_tensor(t2[:], prot[:], sinT[:, tsl], mult)
                    if m < 2:
                        nc.vector.tensor_tensor(qkT[m][:, tsl], t1[:], t2[:],
                                                add)
                    else:
                        # packed [q2|k2]: split to qkT[2]/qkT[3] (cross-part)
                        nc.vector.tensor_tensor(qkT[2][:, tsl], t1[0:64, :],
                                                t2[0:64, :], add)
                        nc.vector.tensor_tensor(qkT[3][:, tsl], t1[64:128, :],
                                                t2[64:128, :], add)

                for i in range(len(chunks)):
                    emit_raw(i)
                    if i >= 2:
                        emit_rope(i - 2)
                for i in (len(chunks) - 2, len(chunks) - 1):
                    emit_rope(i)

                # V projection in natural [t, d] layout
                for t in range(TQ):
                    tsl = slice(128 * t, 128 * (t + 1))
                    pv = qps.tile([128, 256], f32, tag="ps", name=f"pv{t}")
                    for c in range(NCC):
                        nc.tensor.matmul(pv[:], xT[c][:, tsl],
                                         wA[c][:, 384:640], start=(c == 0),
                                         stop=(c == NCC - 1))
                    nc.vector.tensor_copy(
                        v_sb[:, t, :, 0:64],
                        pv[:, 0:192].rearrange("p (h d) -> p h d", d=64))

                qkv_ps_pool.__exit__(None, None, None)
                qkv_pool.__exit__(None, None, None)

                # ========== attention + out projection (interleaved) ==========
                attn_pool = tc.tile_pool(name=f"attn{rep}", bufs=1)
                ap = attn_pool.__enter__()
                attn_ps_pool = tc.tile_pool(name=f"attnps{rep}", bufs=2, space="PSUM")
                aps = attn_ps_pool.__enter__()

                # q/k row views per head: (tile index, partition offset)
                qv = [(0, 0), (0, 64), (2, 0)]
                kv = [(1, 0), (1, 64), (3, 0)]

                for g in range(NGRP):
                    for h in range(HPG):
                        qm, qo = qv[h]
                        km, ko = kv[h]
                        qT = qkT[qm][qo:qo + 64, :]
                        kT = qkT[km][ko:ko + 64, :]
                        nj = (GW // 128) * (g + 1)
                        # pass A: scores + exp (+ causal tri) for every k-chunk
                        pts = []
                        for j in range(nj):
                            dj = j - (GW // 128) * g
                            col0 = 128 * dj if dj >= 0 else 0
                            pscr = aps.tile([128, GW], f32, tag="pscr", bufs=2,
                                            name=f"pscr{g}_{h}_{j}")
                            for s0 in range(col0 - col0 % 512, GW, 512):
                                a0 = max(s0, col0)
                                nc.tensor.matmul(
                                    pscr[:, a0:s0 + 512],
                                    kT[:, 128 * j:128 * (j + 1)],
                                    qT[:, GW * g + a0:GW * g + s0 + 512],
                                    start=True, stop=True)
                            pt = ap.tile([128, GW], f32r, tag="pt", bufs=17,
                                         name=f"pt{g}_{h}_{j}")
                            nc.scalar.activation(pt[:, col0:], pscr[:, col0:],
                                                 Exp, scale=0.125)
                            if dj >= 0:
                                nc.gpsimd.tensor_tensor(
                                    pt[:, col0:col0 + 128],
                                    pt[:, col0:col0 + 128], tri[:], mult)
                            pts.append((pt, col0))
                        # pass B: P^T @ V into two 512-wide accumulators
                        pos = [aps.tile([65, 512], f32, tag="pso", bufs=4,
                                        name=f"po{g}_{h}_{i2}")
                               for i2 in range(GW // 512)]
                        lastw = {}
                        for j in range(nj):
                            _, col0 = pts[j]
                            for s0 in range(col0 - col0 % 512, GW, 512):
                                lastw[s0 // 512] = j
                        for j in range(nj):
                            pt, col0 = pts[j]
                            for s0 in range(col0 - col0 % 512, GW, 512):
                                a0 = max(s0, col0)
                                hv = s0 // 512
                                nc.tensor.matmul(
                                    pos[hv][:, a0 - s0:512], v_sb[:, j, h, :],
                                    pt[:, a0:s0 + 512], start=(j == 0),
                                    stop=(j == lastw[hv]), skip_group_check=True)
                        # normalize per half: attnT = po[0:64] * (1/po[64]),
                        # denominator broadcast across partitions on GPSIMD
                        for hv in range(GW // 512):
                            po = pos[hv]
                            csl = slice(GW * g + 512 * hv, GW * g + 512 * (hv + 1))
                            rc0 = ap.tile([1, 512], f32, tag="rc0", bufs=2,
                                          name=f"rc0{g}_{h}_{hv}")
                            nc.vector.reciprocal(rc0[:], po[64:65, :])
                            pbb = ap.tile([64, 512], f32, tag="pbb", bufs=3,
                                          name=f"pbb{g}_{h}_{hv}")
                            nc.gpsimd.partition_broadcast(pbb[:], rc0[:])
                            dstT, dofs = attn_dst[h]
                            nc.vector.tensor_tensor(dstT[dofs:dofs + D, csl],
                                                    po[0:64, :], pbb[:], mult)

                    # out projection for this g's t-range, from the same pool
                    for t in range((TQ // NGRP) * g, (TQ // NGRP) * (g + 1)):
                        tsl = slice(128 * t, 128 * (t + 1))
                        for c0, cn in ((0, 512), (512, 256)):
                            pout = aps.tile([128, cn], f32, tag="pso", bufs=4,
                                            name=f"pout{t}_{c0}")
                            nc.tensor.matmul(pout[:], attnT_A[:, tsl],
                                             woA[:, c0:c0 + cn], start=True,
                                             stop=False)
                            nc.tensor.matmul(pout[:], attnT_B[:, tsl],
                                             woB[:, c0:c0 + cn], start=False,
                                             stop=True)
                            osb = ap.tile([128, cn], f32, tag=f"osb{c0}", bufs=3,
                                          name=f"osb{t}_{c0}")
                            nc.any.tensor_copy(osb[:], pout[:])
                            dmaq[2 * ((t + (1 if c0 else 0)) % 2)].dma_start(
                                out_d[tsl, c0:c0 + cn], osb[:])

                attn_ps_pool.__exit__(None, None, None)
                attn_pool.__exit__(None, None, None)

    nc.compile()
    return nc


def _host_inputs(x, w_qkv, w_out):
    """Build the 8 per-core input maps."""
    inv_freq = 1.0 / (ROPE_BASE ** (np.arange(0, D, 2, dtype=np.float32) / D))
    t = np.arange(T, dtype=np.float32)
    freqs = t[:, None] * inv_freq[None, :]          # [T, D/2]
    emb = np.concatenate([freqs, freqs], axis=-1)   # [T, D]
    cosT = np.ascontiguousarray(np.cos(emb).T.astype(np.float32))  # [D, T]
    sinT = np.ascontiguousarray(np.sin(emb).T.astype(np.float32))
    cosT2 = np.concatenate([cosT, cosT], axis=0)    # [128, T]
    sinT2 = np.concatenate([sinT, sinT], axis=0)

    # rotate_half permutation as matmul lhsT: rot = R @ q, lhsT = R.T
    R = np.zeros((D, D), np.float32)
    R[0:32, 32:64] = -np.eye(32)
    R[32:64, 0:32] = np.eye(32)
    R2 = np.zeros((128, 128), np.float32)
    R2[0:64, 0:64] = R
    R2[64:128, 64:128] = R
    rT = np.ascontiguousarray(R2.T)

    tri = np.zeros((128, 128), np.float32)
    for kr in range(128):
        tri[kr, kr:] = 1.0

    wq = w_qkv[0:C]
    wk = w_qkv[C:2 * C]
    wv = w_qkv[2 * C:3 * C]

    maps = []
    for core in range(NG):
        b, hg = core // 4, core % 4
        hs = slice(HPG * D * hg, HPG * D * (hg + 1))   # 192 rows of this group
        h2 = HPG * D * hg + 2 * D
        q01 = wq[hs][0:128]                             # [128, C]
        k01 = wk[hs][0:128]
        q2 = wq[h2:h2 + D]
        k2 = wk[h2:h2 + D]
        v3 = wv[hs]                                     # [192, C]
        wA = np.zeros((C, 640), np.float32)
        wA[:, 0:128] = q01.T
        wA[:, 128:256] = k01.T
        wA[:, 256:320] = q2.T
        wA[:, 320:384] = k2.T
        wA[:, 384:576] = v3.T
        wo_h = [w_out[:, HPG * D * hg + D * h: HPG * D * hg + D * (h + 1)].T
                for h in range(HPG)]                    # 3 x [64, C]
        woA = np.concatenate([wo_h[0], wo_h[1]], axis=0)  # [128, C]
        woB = wo_h[2]                                     # [64, C]
        maps.append({
            "xT": np.ascontiguousarray(x[b].T),
            "wA": np.ascontiguousarray(wA),
            "woA": np.ascontiguousarray(woA.astype(np.float32)),
            "woB": np.ascontiguousarray(woB.astype(np.float32)),
            "cosT": cosT2, "sinT": sinT2,
            "rT": rT, "tri": tri,
        })
    return maps


def kernel(x, w_qkv, w_out):
    from concourse.bass_utils import run_bass_kernel_spmd

    if "nc" not in _CACHE:
        _CACHE["nc"] = _build_nc()
    nc = _CACHE["nc"]

    maps = _host_inputs(np.asarray(x, np.float32),
                        np.asarray(w_qkv, np.float32),
                        np.asarray(w_out, np.float32))
    res = run_bass_kernel_spmd(nc, maps, core_ids=list(range(NG))).results
    parts = np.stack([r["out"] for r in res])           # [8, T, C]
    out = np.zeros((B, T, C), np.float32)
    for b in range(B):
        out[b] = parts[4 * b:4 * (b + 1)].sum(axis=0)
    return out



# revision 4
# speedup vs baseline: 1.0922x; 1.0922x over previous
"""Multi-head attention (12 heads, RoPE, causal SDPA) for Trainium2, 8 cores.

Sharding: batch (2) x head-group (4 groups of 3 heads). Each core computes,
for its (batch b, head-group hg): QKV projection for its 3 heads, RoPE,
causal attention, and a partial out-projection [T, C] restricted to its
heads' rows of w_out. The host sums the 4 head-group partials per batch.

All matmul operands are bf16 (PSUM accumulation stays f32), halving DMA
traffic and dodging the f32r narrow-moving-operand penalty. Attention is
computed transposed (S^T[k, q] = K Q^T blocks) so softmax P^T lands in
[k, q] layout. P@V uses P^T blocks as the *stationary* operand and V
[128, 65] as the *moving* operand (65 PE rows per (q-tile, k-chunk) pair
instead of ~128), accumulating o[q, d] plus the softmax denominator (ones
column appended to V) in natural layout. Each accumulator is normalized by
a per-partition reciprocal multiply, transposed back to [d, q] on the PE
(identity matmul), and fed to the out-projection.

Engine budget (cost model): PE ~71us is the bottleneck; ACT carries the
~60us exp stream; DVE all PSUM->SBUF copies (GPSIMD cannot access PSUM on
HW) plus RoPE/normalize elementwise; Pool carries SBUF-only elementwise +
a few DMA issues. The emission order software-pipelines QKV chunks,
scores, exp, P@V, transposes and the out-projection, with g1 score passes
(pscr-throttled to exp pace) carrying independent V-proj/PV/epilogue work
as fillers, so PE and ACT stay saturated. A dozen throwaway matmuls warm
the PE p-state ramp while the first DMAs land. DMA issue occupies the
issuing engine ~500ns, so issues are spread over SP/ACT/Pool by phase.

Device-side layouts (T = 2048, C = 768, D = 64 per head):
  xT   [768, 2048]  x[b] transposed (c on partitions)          bf16
  wA   [768, 576]   packed lhsT weights: cols 0:128 [q0|q1], 128:256
                    [k0|k1], 256:320 q2, 320:384 k2, 384:576 w_v  bf16
  woA  [128, 768]   w_out rows for heads 0,1; woB [64, 768] head 2
  cosT/sinT [128, 2048]  RoPE tables transposed, stacked twice  bf16
  rT   [128, 128]   rotate_half as matmul lhsT                  bf16
  tri  [128, 128]   tri[kr, qc] = 1 if qc >= kr (causal, S^T)   bf16
  ident [128, 128]  identity (PE transpose permutation operand) bf16
  out  [2048, 768]  partial output, bf16 (host sums in f32)
"""
import numpy as np

B, T, C, H, D = 2, 2048, 768, 12, 64
HPG = 3                    # heads per group
NG = B * (H // HPG)        # 8 cores
ROPE_BASE = 10000.0
TQ = T // 128              # 16 t-tiles
NCC = C // 128             # 6 contraction chunks
GW = 1024                  # attention q-group width
NGRP = T // GW             # 2 q-groups
GT = GW // 128             # 8 q-tiles per group

_CACHE = {}


def _build_nc(reps=1):
    from concourse import bacc, tile, mybir

    f32 = mybir.dt.float32
    bf16 = mybir.dt.bfloat16
    Exp = mybir.ActivationFunctionType.Exp
    mult = mybir.AluOpType.mult
    add = mybir.AluOpType.add

    nc = bacc.Bacc("TRN2", target_bir_lowering=False, debug=False,
                   num_devices=NG)

    xT_d = nc.dram_tensor("xT", [C, T], bf16, kind="ExternalInput").ap()
    wA_d = nc.dram_tensor("wA", [C, 576], bf16, kind="ExternalInput").ap()
    woA_d = nc.dram_tensor("woA", [2 * D, C], bf16, kind="ExternalInput").ap()
    woB_d = nc.dram_tensor("woB", [D, C], bf16, kind="ExternalInput").ap()
    cosT_d = nc.dram_tensor("cosT", [128, T], bf16, kind="ExternalInput").ap()
    sinT_d = nc.dram_tensor("sinT", [128, T], bf16, kind="ExternalInput").ap()
    rT_d = nc.dram_tensor("rT", [128, 128], bf16, kind="ExternalInput").ap()
    tri_d = nc.dram_tensor("tri", [128, 128], bf16, kind="ExternalInput").ap()
    ident_d = nc.dram_tensor("ident", [128, 128], bf16,
                             kind="ExternalInput").ap()
    out_d = nc.dram_tensor("out", [T, C], bf16, kind="ExternalOutput").ap()

    with tile.TileContext(nc) as tc:
      with tc.tile_pool(name="persist", bufs=1) as pp, \
           tc.tile_pool(name="work", bufs=1) as wp, \
           tc.tile_pool(name="psum", bufs=1, space="PSUM") as aps:
        for rep in range(reps):
            dmaq = [nc.sync, nc.gpsimd]

            # ---- persistent constants / state ----
            woA = pp.tile([2 * D, C], bf16, tag="woA")
            woB = pp.tile([D, C], bf16, tag="woB")
            tri = pp.tile([128, 128], bf16, tag="tri")
            ident = pp.tile([128, 128], bf16, tag="ident")
            rT = pp.tile([128, 128], bf16, tag="rT")
            cosT = pp.tile([128, T], bf16, tag="cosT")
            sinT = pp.tile([128, T], bf16, tag="sinT")

            qk_rows = [128, 128, 64, 64]      # [q0|q1], [k0|k1], q2, k2
            qkT = [pp.tile([qk_rows[m], T], bf16, tag=f"qkT{m}",
                           name=f"qkT{m}") for m in range(4)]
            v_sb = pp.tile([128, TQ, HPG, 65], bf16, tag="v_sb")
            attnTa = pp.tile([2 * D, T], bf16, tag="attnTa")
            attnTb = pp.tile([D, T], bf16, tag="attnTb")
            attn01 = [pp.tile([128, 128], bf16, tag=f"attn01_{i}",
                              name=f"attn01_{i}") for i in range(GT)]
            attn2 = [pp.tile([128, D], bf16, tag=f"attn2_{i}",
                             name=f"attn2_{i}") for i in range(GT)]

            # ---- work tiles ----
            xT = [wp.tile([128, T], bf16, tag=f"xT{c}", name=f"xT{c}")
                  for c in range(NCC)]
            wA = [wp.tile([128, 576], bf16, tag=f"wA{c}", name=f"wA{c}")
                  for c in range(NCC)]

            # ---- input DMA ----
            # DMA issue occupies the issuing engine's queue ~500ns each, so
            # Pool/ACT never issue; SP carries most, DVE the critical xT
            # halves. Pieces are consolidated to cut issue count.
            # ACT and Pool are idle at the start — they issue the critical
            # xT n0/n1 halves while SP walks the rest in priority order.
            for c in range(NCC):
                dmaq[0].dma_start(wA[c][:, 0:256],
                                  wA_d[128 * c:128 * (c + 1), 0:256])
                nc.scalar.dma_start(xT[c][:, 0:512],
                                    xT_d[128 * c:128 * (c + 1), 0:512])
                nc.gpsimd.dma_start(xT[c][:, 512:1024],
                                    xT_d[128 * c:128 * (c + 1), 512:1024])
            dmaq[0].dma_start(rT[:], rT_d[:])
            dmaq[0].dma_start(cosT[:, 0:512], cosT_d[:, 0:512])
            dmaq[0].dma_start(sinT[:, 0:512], sinT_d[:, 0:512])
            dmaq[0].dma_start(cosT[:, 512:1024], cosT_d[:, 512:1024])
            dmaq[0].dma_start(sinT[:, 512:1024], sinT_d[:, 512:1024])
            for c in range(NCC):
                nc.scalar.dma_start(wA[c][:, 256:384],
                                    wA_d[128 * c:128 * (c + 1), 256:384])
            for c in range(NCC):
                dmaq[0].dma_start(xT[c][:, 1024:2048],
                                  xT_d[128 * c:128 * (c + 1), 1024:2048])
            dmaq[0].dma_start(cosT[:, 1024:2048], cosT_d[:, 1024:2048])
            dmaq[0].dma_start(sinT[:, 1024:2048], sinT_d[:, 1024:2048])
            for c in range(NCC):
                dmaq[0].dma_start(wA[c][:, 384:576],
                                  wA_d[128 * c:128 * (c + 1), 384:576])
            dmaq[0].dma_start(tri[:], tri_d[:])
            dmaq[0].dma_start(ident[:], ident_d[:])
            dmaq[0].dma_start(woA[:], woA_d[:])
            dmaq[0].dma_start(woB[:], woB_d[:])

            nc.vector.memset(v_sb[:, :, :, 64:65], 1.0)

            # ================= QKV machinery =================
            qk_cols = [(0, 128), (128, 256), (256, 384)]
            raws = {}

            def emit_praw(key, m, n, copy_eng):
                c0, c1 = qk_cols[m]
                tsl = slice(512 * n, 512 * (n + 1))
                praw = aps.tile([128, 512], f32, tag="bank", bufs=3,
                                name=f"praw{key}")
                for c in range(NCC):
                    nc.tensor.matmul(praw[:], wA[c][:, c0:c1], xT[c][:, tsl],
                                     start=(c == 0), stop=(c == NCC - 1))
                raw = wp.tile([128, 512], bf16, tag="raw", bufs=4,
                              name=f"raw{key}")
                if copy_eng == "act":
                    nc.scalar.copy(raw[:], praw[:])
                else:
                    nc.gpsimd.tensor_copy(raw[:], praw[:])
                raws[key] = raw

            def emit_rope(key, m, n):
                tsl = slice(512 * n, 512 * (n + 1))
                raw = raws.pop(key)
                prot = aps.tile([128, 512], f32, tag="bank", bufs=3,
                                name=f"prot{key}")
                nc.tensor.matmul(prot[:], rT[:], raw[:], start=True, stop=True)
                t1 = wp.tile([128, 512], bf16, tag="t1", bufs=3,
                             name=f"t1_{key}")
                nc.vector.tensor_tensor(t1[:], raw[:], cosT[:, tsl], mult)
                t2 = wp.tile([128, 512], bf16, tag="t2", bufs=3,
                             name=f"t2_{key}")
                nc.vector.tensor_tensor(t2[:], prot[:], sinT[:, tsl], mult)
                if m < 2:
                    nc.vector.tensor_tensor(qkT[m][:, tsl], t1[:], t2[:], add)
                else:
                    nc.vector.tensor_tensor(qkT[2][:, tsl], t1[0:64, :],
                                            t2[0:64, :], add)
                    nc.vector.tensor_tensor(qkT[3][:, tsl], t1[64:128, :],
                                            t2[64:128, :], add)

            def emit_v(t):
                tsl = slice(128 * t, 128 * (t + 1))
                pv = aps.tile([128, 512], f32, tag="bank", bufs=3,
                              name=f"pv{t}")
                for c in range(NCC):
                    nc.tensor.matmul(pv[:, 0:192], xT[c][:, tsl],
                                     wA[c][:, 384:576], start=(c == 0),
                                     stop=(c == NCC - 1))
                nc.gpsimd.tensor_copy(
                    v_sb[:, t, :, 0:64],
                    pv[:, 0:192].rearrange("p (h d) -> p h d", d=64))

            def emit_qk_chunks(chunks, copy_eng="pool"):
                # rot lags praw by one chunk so the PE never waits on the
                # psum->sbuf copy
                prev = None
                for (m, n) in chunks:
                    key = (m, n)
                    emit_praw(key, m, n, copy_eng)
                    if prev is not None:
                        emit_rope(prev, *prev)
                    prev = key
                emit_rope(prev, *prev)

            # ================= attention machinery =================
            # q/k row views per head: (tile index, partition offset)
            qv = [(0, 0), (0, 64), (2, 0)]
            kv = [(1, 0), (1, 64), (3, 0)]

            def emit_scores(g, h, fillers=None):
                """Pass A: S^T blocks + exp -> pt tiles. Returns pts list.

                The pscr double-buffer paces this loop at ACT exp speed, so
                `fillers` (independent PE work items) are spliced between
                k-chunks to keep the PE busy during the forced waits."""
                qm, qo = qv[h]
                km, ko = kv[h]
                qT = qkT[qm][qo:qo + 64, :]
                kT = qkT[km][ko:ko + 64, :]
                nj = GT * (g + 1)
                pts = []
                for j in range(nj):
                    dj = j - GT * g
                    col0 = 128 * dj if dj >= 0 else 0
                    pscr = aps.tile([128, GW], f32, tag="pscr", bufs=2,
                                    name=f"pscr{g}_{h}_{j}")
                    for s0 in range(col0 - col0 % 512, GW, 512):
                        a0 = max(s0, col0)
                        nc.tensor.matmul(
                            pscr[:, a0:s0 + 512],
                            kT[:, 128 * j:128 * (j + 1)],
                            qT[:, GW * g + a0:GW * g + s0 + 512],
                            start=True, stop=True)
                    pt = wp.tile([128, GW], bf16, tag="pt", bufs=48,
                                 name=f"pt{g}_{h}_{j}")
                    nc.scalar.activation(pt[:, col0:], pscr[:, col0:],
                                         Exp, scale=0.125)
                    pts.append((pt, col0))
                    if fillers:
                        fillers.pop(0)()
                while fillers:
                    fillers.pop(0)()
                return pts

            def emit_scores_early(h):
                """g=0 pass A, k-chunks 0..3, q cols [col0:512] only —
                unblocks the ACT exp stream before qkT n=1 exists."""
                qm, qo = qv[h]
                km, ko = kv[h]
                qT = qkT[qm][qo:qo + 64, :]
                kT = qkT[km][ko:ko + 64, :]
                pts = []
                for j in range(4):
                    col0 = 128 * j
                    pscr = aps.tile([128, 512], f32, tag="bank", bufs=3,
                                    name=f"pscrE{h}_{j}")
                    nc.tensor.matmul(pscr[:, col0:512],
                                     kT[:, 128 * j:128 * (j + 1)],
                                     qT[:, col0:512], start=True, stop=True)
                    pt = wp.tile([128, GW], bf16, tag="pt", bufs=48,
                                 name=f"ptE{h}_{j}")
                    nc.scalar.activation(pt[:, col0:512], pscr[:, col0:512],
                                         Exp, scale=0.125)
                    pts.append((pt, col0))
                return pts

            def emit_scores_late(h, pts):
                """g=0 pass A remainder: cols [512:1024] for k-chunks 0..3,
                full rows for k-chunks 4..7."""
                qm, qo = qv[h]
                km, ko = kv[h]
                qT = qkT[qm][qo:qo + 64, :]
                kT = qkT[km][ko:ko + 64, :]
                for j in range(GT):
                    col0 = 128 * j
                    lo = max(col0, 512)
                    pscr = aps.tile([128, GW], f32, tag="pscr", bufs=2,
                                    name=f"pscrL{h}_{j}")
                    for s0 in range(lo - lo % 512, GW, 512):
                        a0 = max(s0, lo)
                        nc.tensor.matmul(
                            pscr[:, a0:s0 + 512],
                            kT[:, 128 * j:128 * (j + 1)],
                            qT[:, a0:s0 + 512], start=True, stop=True)
                    if j < 4:
                        pt, _ = pts[j]
                    else:
                        pt = wp.tile([128, GW], bf16, tag="pt", bufs=48,
                                     name=f"ptL{h}_{j}")
                        pts.append((pt, col0))
                    nc.scalar.activation(pt[:, lo:], pscr[:, lo:],
                                         Exp, scale=0.125)
                return pts

            def _pv_finish(g, h, i, acc):
                rc = wp.tile([128, 1], f32, tag="rc", bufs=4,
                             name=f"rc{g}_{h}_{i}")
                nc.vector.reciprocal(rc[:], acc[:, 64:65])
                if h < 2:
                    dst = attn01[i][:, 64 * h:64 * (h + 1)]
                else:
                    dst = attn2[i][:]
                nc.vector.tensor_scalar_mul(dst, acc[:, 0:64], rc[:])

            def emit_pv_tile(g, h, i, pts):
                """P^T@V for q-tile i of group g: acc[q, 0:64]=o, [64]=denom."""
                jmax = GT * g + i
                pt_d, _ = pts[jmax]
                dcol = 128 * i
                nc.vector.tensor_tensor(pt_d[:, dcol:dcol + 128],
                                        pt_d[:, dcol:dcol + 128], tri[:],
                                        mult)
                acc = aps.tile([128, 512], f32, tag="bank", bufs=3,
                               name=f"acc{g}_{h}_{i}")
                for j in range(jmax + 1):
                    ptj, _ = pts[j]
                    nc.tensor.matmul(acc[:, 0:65],
                                     ptj[:, 128 * i:128 * (i + 1)],
                                     v_sb[:, j, h, :],
                                     start=(j == 0), stop=(j == jmax),
                                     skip_group_check=True)
                _pv_finish(g, h, i, acc)

            def emit_t01(g, i):
                ptr = aps.tile([128, 128], bf16, tag="ptr", bufs=1,
                               name=f"ptr01_{g}_{i}")
                nc.tensor.transpose(ptr[:], attn01[i][:], ident[:])
                csl = slice(GW * g + 128 * i, GW * g + 128 * (i + 1))
                nc.vector.tensor_copy(attnTa[:, csl], ptr[:])

            def emit_t2(g, i):
                ptr = aps.tile([128, 128], bf16, tag="ptr", bufs=1,
                               name=f"ptr2_{g}_{i}")
                nc.tensor.transpose(ptr[0:64, :], attn2[i][:], ident[:])
                csl = slice(GW * g + 128 * i, GW * g + 128 * (i + 1))
                nc.vector.tensor_copy(attnTb[:, csl], ptr[0:64, :])

            def emit_outproj(g, i):
                t = GT * g + i
                tsl = slice(128 * t, 128 * (t + 1))
                for c0, cn in ((0, 512), (512, 256)):
                    pout = aps.tile([128, 512], f32, tag="bank", bufs=3,
                                    name=f"pout{t}_{c0}")
                    nc.tensor.matmul(pout[:, 0:cn], attnTa[:, tsl],
                                     woA[:, c0:c0 + cn], start=True,
                                     stop=False)
                    nc.tensor.matmul(pout[:, 0:cn], attnTb[:, tsl],
                                     woB[:, c0:c0 + cn], start=False,
                                     stop=True)
                    osb = wp.tile([128, cn], bf16, tag=f"osb{c0}", bufs=3,
                                  name=f"osb{t}_{c0}")
                    # Pool has slack everywhere; keeps the epilogue off the
                    # DVE chain and the ACT exp stream
                    nc.gpsimd.tensor_copy(osb[:], pout[:, 0:cn])
                    # ACT is idle in the tail — split the final DMA issues
                    if g == 1 and i >= 3 and c0 == 0:
                        nc.scalar.dma_start(out_d[tsl, c0:c0 + cn], osb[:])
                    else:
                        dmaq[0].dma_start(out_d[tsl, c0:c0 + cn], osb[:])

            def emit_pv_block(g, h, pts, with_epilogue):
                """B pass; h==2 interleaves T01/T2/outproj epilogue."""
                for i in range(GT):
                    emit_pv_tile(g, h, i, pts)
                    if with_epilogue:
                        emit_t01(g, i)
                        if i >= 1:
                            emit_t2(g, i - 1)
                        if i >= 2:
                            emit_outproj(g, i - 2)
                if with_epilogue:
                    emit_t2(g, GT - 1)
                    emit_outproj(g, GT - 2)
                    emit_outproj(g, GT - 1)

            # ================= emission schedule =================
            # Software-pipelined so the ACT exp stream (the second
            # bottleneck) starts early and never waits long on scores.
            emit_qk_chunks([(0, 0), (1, 0)])
            pts00 = emit_scores_early(0)
            pts01 = emit_scores_early(1)
            emit_qk_chunks([(0, 1), (1, 1)])
            pts00 = emit_scores_late(0, pts00)
            pts01 = emit_scores_late(1, pts01)
            emit_qk_chunks([(2, 0), (2, 1)])
            pts02 = emit_scores(0, 2)
            emit_qk_chunks([(m, n) for n in (2, 3) for m in (0, 1, 2)])
            # g1 score passes are exp-paced; fill their PE waits with the
            # independent V-proj, g0 PV blocks, and the g0 epilogue.
            def mk(f, *a):
                return lambda: f(*a)

            fil0 = [mk(emit_v, t) for t in range(8)]
            fil0 += [mk(emit_pv_tile, 0, 0, i, pts00) for i in range(GT)]
            pts10 = emit_scores(1, 0, fil0)

            fil1 = [mk(emit_pv_tile, 0, 1, i, pts01) for i in range(GT)]
            fil1 += [mk(emit_v, t) for t in range(8, 16)]
            pts11 = emit_scores(1, 1, fil1)

            def mk_epi0(i):
                def f():
                    emit_pv_tile(0, 2, i, pts02)
                    emit_t01(0, i)
                    if i >= 1:
                        emit_t2(0, i - 1)
                    if i >= 2:
                        emit_outproj(0, i - 2)
                    if i == GT - 1:
                        emit_t2(0, GT - 1)
                        emit_outproj(0, GT - 2)
                        emit_outproj(0, GT - 1)
                return f

            fil2 = [mk_epi0(i) for i in range(GT)]
            fil2 += [mk(emit_pv_tile, 1, 0, i, pts10) for i in range(GT)]
            pts12 = emit_scores(1, 2, fil2)

            emit_pv_block(1, 1, pts11, False)
            emit_pv_block(1, 2, pts12, True)

    nc.compile()
    return nc


def _host_inputs(x, w_qkv, w_out):
    """Build the 8 per-core input maps (bf16)."""
    import ml_dtypes
    bf = ml_dtypes.bfloat16

    inv_freq = 1.0 / (ROPE_BASE ** (np.arange(0, D, 2, dtype=np.float32) / D))
    t = np.arange(T, dtype=np.float32)
    freqs = t[:, None] * inv_freq[None, :]          # [T, D/2]
    emb = np.concatenate([freqs, freqs], axis=-1)   # [T, D]
    cosT = np.ascontiguousarray(np.cos(emb).T)      # [D, T]
    sinT = np.ascontiguousarray(np.sin(emb).T)
    cosT2 = np.concatenate([cosT, cosT], axis=0).astype(bf)   # [128, T]
    sinT2 = np.concatenate([sinT, sinT], axis=0).astype(bf)

    # rotate_half permutation as matmul lhsT: rot = R @ q, lhsT = R.T
    R = np.zeros((D, D), np.float32)
    R[0:32, 32:64] = -np.eye(32)
    R[32:64, 0:32] = np.eye(32)
    R2 = np.zeros((128, 128), np.float32)
    R2[0:64, 0:64] = R
    R2[64:128, 64:128] = R
    rT = np.ascontiguousarray(R2.T).astype(bf)

    tri = np.zeros((128, 128), np.float32)
    for kr in range(128):
        tri[kr, kr:] = 1.0
    tri = tri.astype(bf)
    ident = np.eye(128, dtype=np.float32).astype(bf)

    wq = w_qkv[0:C]
    wk = w_qkv[C:2 * C]
    wv = w_qkv[2 * C:3 * C]

    maps = []
    for core in range(NG):
        b, hg = core // 4, core % 4
        hs = slice(HPG * D * hg, HPG * D * (hg + 1))   # 192 rows of this group
        h2 = HPG * D * hg + 2 * D
        q01 = wq[hs][0:128]                             # [128, C]
        k01 = wk[hs][0:128]
        q2 = wq[h2:h2 + D]
        k2 = wk[h2:h2 + D]
        v3 = wv[hs]                                     # [192, C]
        wA = np.zeros((C, 576), np.float32)
        wA[:, 0:128] = q01.T
        wA[:, 128:256] = k01.T
        wA[:, 256:320] = q2.T
        wA[:, 320:384] = k2.T
        wA[:, 384:576] = v3.T
        wo_h = [w_out[:, HPG * D * hg + D * h: HPG * D * hg + D * (h + 1)].T
                for h in range(HPG)]                    # 3 x [64, C]
        woA = np.concatenate([wo_h[0], wo_h[1]], axis=0)  # [128, C]
        woB = wo_h[2]                                     # [64, C]
        maps.append({
            "xT": np.ascontiguousarray(x[b].T).astype(bf),
            "wA": np.ascontiguousarray(wA).astype(bf),
            "woA": np.ascontiguousarray(woA).astype(bf),
            "woB": np.ascontiguousarray(woB).astype(bf),
            "cosT": cosT2, "sinT": sinT2,
            "rT": rT, "tri": tri, "ident": ident,
        })
    return maps


def kernel(x, w_qkv, w_out):
    from concourse.bass_utils import run_bass_kernel_spmd

    if "nc" not in _CACHE:
        _CACHE["nc"] = _build_nc()
    nc = _CACHE["nc"]

    maps = _host_inputs(np.asarray(x, np.float32),
                        np.asarray(w_qkv, np.float32),
                        np.asarray(w_out, np.float32))
    res = run_bass_kernel_spmd(nc, maps, core_ids=list(range(NG))).results
    parts = np.stack([np.asarray(r["out"], dtype=np.float32)
                      for r in res])                    # [8, T, C]
    out = np.zeros((B, T, C), np.float32)
    for b in range(B):
        out[b] = parts[4 * b:4 * (b + 1)].sum(axis=0)
    return out


# revision 5
# speedup vs baseline: 1.1031x; 1.0100x over previous
"""Multi-head attention (12 heads, RoPE, causal SDPA) for Trainium2, 8 cores.

Sharding: batch (2) x head-group (4 groups of 3 heads). Each core computes,
for its (batch b, head-group hg): QKV projection for its 3 heads, RoPE,
causal attention, and a partial out-projection [T, C] restricted to its
heads' rows of w_out. The host sums the 4 head-group partials per batch.

All matmul operands are bf16 (PSUM accumulation stays f32), halving DMA
traffic and dodging the f32r narrow-moving-operand penalty. Attention is
computed transposed (S^T[k, q] = K Q^T blocks) so softmax P^T lands in
[k, q] layout. P@V uses P^T blocks as the *stationary* operand and V
[128, 65] as the *moving* operand (65 PE rows per (q-tile, k-chunk) pair
instead of ~128), accumulating o[q, d] plus the softmax denominator (ones
column appended to V) in natural layout. Each accumulator is normalized by
a per-partition reciprocal multiply, transposed back to [d, q] on the PE
(identity matmul), and fed to the out-projection.

Engine budget (cost model): PE ~71us is the bottleneck; ACT carries the
~60us exp stream; DVE all PSUM->SBUF copies (GPSIMD cannot access PSUM on
HW) plus RoPE/normalize elementwise; Pool carries SBUF-only elementwise +
a few DMA issues. The emission order software-pipelines QKV chunks,
scores, exp, P@V, transposes and the out-projection, with g1 score passes
(pscr-throttled to exp pace) carrying independent V-proj/PV/epilogue work
as fillers, so PE and ACT stay saturated. DMA issue occupies the issuing
engine ~500ns, so issues are spread over SP/ACT/Pool by phase. The reps
share one pool scope so back-to-back invocations pipeline: the next rep's
DMA and projections overlap the previous rep's attention tail (the
steady-state marginal rep is ~93%-PE-bound at ~71us of PE work).

Device-side layouts (T = 2048, C = 768, D = 64 per head):
  xT   [768, 2048]  x[b] transposed (c on partitions)          bf16
  wA   [768, 576]   packed lhsT weights: cols 0:128 [q0|q1], 128:256
                    [k0|k1], 256:320 q2, 320:384 k2, 384:576 w_v  bf16
  woA  [128, 768]   w_out rows for heads 0,1; woB [64, 768] head 2
  cosT/sinT [128, 2048]  RoPE tables transposed, stacked twice  bf16
  rT   [128, 128]   rotate_half as matmul lhsT                  bf16
  tri  [128, 128]   tri[kr, qc] = 1 if qc >= kr (causal, S^T)   bf16
  ident [128, 128]  identity (PE transpose permutation operand) bf16
  out  [2048, 768]  partial output, bf16 (host sums in f32)
"""
import numpy as np

B, T, C, H, D = 2, 2048, 768, 12, 64
HPG = 3                    # heads per group
NG = B * (H // HPG)        # 8 cores
ROPE_BASE = 10000.0
TQ = T // 128              # 16 t-tiles
NCC = C // 128             # 6 contraction chunks
GW = 1024                  # attention q-group width
NGRP = T // GW             # 2 q-groups
GT = GW // 128             # 8 q-tiles per group

_CACHE = {}


def _build_nc(reps=1):
    from concourse import bacc, tile, mybir

    f32 = mybir.dt.float32
    bf16 = mybir.dt.bfloat16
    Exp = mybir.ActivationFunctionType.Exp
    mult = mybir.AluOpType.mult
    add = mybir.AluOpType.add

    nc = bacc.Bacc("TRN2", target_bir_lowering=False, debug=False,
                   num_devices=NG)

    xT_d = nc.dram_tensor("xT", [C, T], bf16, kind="ExternalInput").ap()
    wA_d = nc.dram_tensor("wA", [C, 576], bf16, kind="ExternalInput").ap()
    woA_d = nc.dram_tensor("woA", [2 * D, C], bf16, kind="ExternalInput").ap()
    woB_d = nc.dram_tensor("woB", [D, C], bf16, kind="ExternalInput").ap()
    cosT_d = nc.dram_tensor("cosT", [128, T], bf16, kind="ExternalInput").ap()
    sinT_d = nc.dram_tensor("sinT", [128, T], bf16, kind="ExternalInput").ap()
    rT_d = nc.dram_tensor("rT", [128, 128], bf16, kind="ExternalInput").ap()
    tri_d = nc.dram_tensor("tri", [128, 128], bf16, kind="ExternalInput").ap()
    ident_d = nc.dram_tensor("ident", [128, 128], bf16,
                             kind="ExternalInput").ap()
    out_d = nc.dram_tensor("out", [T, C], bf16, kind="ExternalOutput").ap()

    with tile.TileContext(nc) as tc:
      with tc.tile_pool(name="persist", bufs=1) as pp, \
           tc.tile_pool(name="work", bufs=1) as wp, \
           tc.tile_pool(name="psum", bufs=1, space="PSUM") as aps:
        for rep in range(reps):
            dmaq = [nc.sync, nc.gpsimd]

            # ---- persistent constants / state ----
            woA = pp.tile([2 * D, C], bf16, tag="woA")
            woB = pp.tile([D, C], bf16, tag="woB")
            tri = pp.tile([128, 128], bf16, tag="tri")
            ident = pp.tile([128, 128], bf16, tag="ident")
            rT = pp.tile([128, 128], bf16, tag="rT")
            cosT = pp.tile([128, T], bf16, tag="cosT")
            sinT = pp.tile([128, T], bf16, tag="sinT")

            qk_rows = [128, 128, 64, 64]      # [q0|q1], [k0|k1], q2, k2
            qkT = [pp.tile([qk_rows[m], T], bf16, tag=f"qkT{m}",
                           name=f"qkT{m}") for m in range(4)]
            v_sb = pp.tile([128, TQ, HPG, 65], bf16, tag="v_sb")
            attnTa = pp.tile([2 * D, T], bf16, tag="attnTa")
            attnTb = pp.tile([D, T], bf16, tag="attnTb")
            attn01 = [pp.tile([128, 128], bf16, tag=f"attn01_{i}",
                              name=f"attn01_{i}") for i in range(GT)]
            attn2 = [pp.tile([128, D], bf16, tag=f"attn2_{i}",
                             name=f"attn2_{i}") for i in range(GT)]

            # ---- work tiles ----
            xT = [wp.tile([128, T], bf16, tag=f"xT{c}", name=f"xT{c}")
                  for c in range(NCC)]
            wA = [wp.tile([128, 576], bf16, tag=f"wA{c}", name=f"wA{c}")
                  for c in range(NCC)]

            # ---- input DMA ----
            # DMA issue occupies the issuing engine's queue ~500ns each, so
            # Pool/ACT never issue; SP carries most, DVE the critical xT
            # halves. Pieces are consolidated to cut issue count.
            # ACT and Pool are idle at the start — they issue the critical
            # xT n0/n1 halves while SP walks the rest in priority order.
            for c in range(NCC):
                dmaq[0].dma_start(wA[c][:, 0:256],
                                  wA_d[128 * c:128 * (c + 1), 0:256])
                nc.scalar.dma_start(xT[c][:, 0:512],
                                    xT_d[128 * c:128 * (c + 1), 0:512])
                nc.gpsimd.dma_start(xT[c][:, 512:1024],
                                    xT_d[128 * c:128 * (c + 1), 512:1024])
            dmaq[0].dma_start(rT[:], rT_d[:])
            dmaq[0].dma_start(cosT[:, 0:512], cosT_d[:, 0:512])
            dmaq[0].dma_start(sinT[:, 0:512], sinT_d[:, 0:512])
            dmaq[0].dma_start(cosT[:, 512:1024], cosT_d[:, 512:1024])
            dmaq[0].dma_start(sinT[:, 512:1024], sinT_d[:, 512:1024])
            for c in range(NCC):
                nc.scalar.dma_start(wA[c][:, 256:384],
                                    wA_d[128 * c:128 * (c + 1), 256:384])
            for c in range(NCC):
                dmaq[0].dma_start(xT[c][:, 1024:2048],
                                  xT_d[128 * c:128 * (c + 1), 1024:2048])
            dmaq[0].dma_start(cosT[:, 1024:2048], cosT_d[:, 1024:2048])
            dmaq[0].dma_start(sinT[:, 1024:2048], sinT_d[:, 1024:2048])
            for c in range(NCC):
                dmaq[0].dma_start(wA[c][:, 384:576],
                                  wA_d[128 * c:128 * (c + 1), 384:576])
            dmaq[0].dma_start(tri[:], tri_d[:])
            dmaq[0].dma_start(ident[:], ident_d[:])
            dmaq[0].dma_start(woA[:], woA_d[:])
            dmaq[0].dma_start(woB[:], woB_d[:])

            nc.vector.memset(v_sb[:, :, :, 64:65], 1.0)

            # ================= QKV machinery =================
            qk_cols = [(0, 128), (128, 256), (256, 384)]
            raws = {}

            def emit_praw(key, m, n, copy_eng):
                c0, c1 = qk_cols[m]
                tsl = slice(512 * n, 512 * (n + 1))
                praw = aps.tile([128, 512], f32, tag="bank", bufs=3,
                                name=f"praw{key}")
                for c in range(NCC):
                    nc.tensor.matmul(praw[:], wA[c][:, c0:c1], xT[c][:, tsl],
                                     start=(c == 0), stop=(c == NCC - 1))
                raw = wp.tile([128, 512], bf16, tag="raw", bufs=4,
                              name=f"raw{key}")
                if copy_eng == "act":
                    nc.scalar.copy(raw[:], praw[:])
                else:
                    nc.gpsimd.tensor_copy(raw[:], praw[:])
                raws[key] = raw

            def emit_rope(key, m, n):
                tsl = slice(512 * n, 512 * (n + 1))
                raw = raws.pop(key)
                prot = aps.tile([128, 512], f32, tag="bank", bufs=3,
                                name=f"prot{key}")
                nc.tensor.matmul(prot[:], rT[:], raw[:], start=True, stop=True)
                t1 = wp.tile([128, 512], bf16, tag="t1", bufs=3,
                             name=f"t1_{key}")
                nc.vector.tensor_tensor(t1[:], raw[:], cosT[:, tsl], mult)
                t2 = wp.tile([128, 512], bf16, tag="t2", bufs=3,
                             name=f"t2_{key}")
                nc.vector.tensor_tensor(t2[:], prot[:], sinT[:, tsl], mult)
                if m < 2:
                    nc.vector.tensor_tensor(qkT[m][:, tsl], t1[:], t2[:], add)
                else:
                    nc.vector.tensor_tensor(qkT[2][:, tsl], t1[0:64, :],
                                            t2[0:64, :], add)
                    nc.vector.tensor_tensor(qkT[3][:, tsl], t1[64:128, :],
                                            t2[64:128, :], add)

            def emit_v(t):
                tsl = slice(128 * t, 128 * (t + 1))
                pv = aps.tile([128, 512], f32, tag="bank", bufs=3,
                              name=f"pv{t}")
                for c in range(NCC):
                    nc.tensor.matmul(pv[:, 0:192], xT[c][:, tsl],
                                     wA[c][:, 384:576], start=(c == 0),
                                     stop=(c == NCC - 1))
                nc.gpsimd.tensor_copy(
                    v_sb[:, t, :, 0:64],
                    pv[:, 0:192].rearrange("p (h d) -> p h d", d=64))

            def emit_qk_chunks(chunks, copy_eng="pool"):
                # rot lags praw by one chunk so the PE never waits on the
                # psum->sbuf copy
                prev = None
                for (m, n) in chunks:
                    key = (m, n)
                    emit_praw(key, m, n, copy_eng)
                    if prev is not None:
                        emit_rope(prev, *prev)
                    prev = key
                emit_rope(prev, *prev)

            # ================= attention machinery =================
            # q/k row views per head: (tile index, partition offset)
            qv = [(0, 0), (0, 64), (2, 0)]
            kv = [(1, 0), (1, 64), (3, 0)]

            def emit_scores(g, h, fillers=None):
                """Pass A: S^T blocks + exp -> pt tiles. Returns pts list.

                The pscr double-buffer paces this loop at ACT exp speed, so
                `fillers` (independent PE work items) are spliced between
                k-chunks to keep the PE busy during the forced waits."""
                qm, qo = qv[h]
                km, ko = kv[h]
                qT = qkT[qm][qo:qo + 64, :]
                kT = qkT[km][ko:ko + 64, :]
                nj = GT * (g + 1)
                pts = []
                for j in range(nj):
                    dj = j - GT * g
                    col0 = 128 * dj if dj >= 0 else 0
                    pscr = aps.tile([128, GW], f32, tag="pscr", bufs=2,
                                    name=f"pscr{g}_{h}_{j}")
                    for s0 in range(col0 - col0 % 512, GW, 512):
                        a0 = max(s0, col0)
                        nc.tensor.matmul(
                            pscr[:, a0:s0 + 512],
                            kT[:, 128 * j:128 * (j + 1)],
                            qT[:, GW * g + a0:GW * g + s0 + 512],
                            start=True, stop=True)
                    pt = wp.tile([128, GW], bf16, tag="pt", bufs=48,
                                 name=f"pt{g}_{h}_{j}")
                    nc.scalar.activation(pt[:, col0:], pscr[:, col0:],
                                         Exp, scale=0.125)
                    pts.append((pt, col0))
                    if fillers:
                        fillers.pop(0)()
                while fillers:
                    fillers.pop(0)()
                return pts

            def emit_scores_early(h):
                """g=0 pass A, k-chunks 0..3, q cols [col0:512] only —
                unblocks the ACT exp stream before qkT n=1 exists."""
                qm, qo = qv[h]
                km, ko = kv[h]
                qT = qkT[qm][qo:qo + 64, :]
                kT = qkT[km][ko:ko + 64, :]
                pts = []
                for j in range(4):
                    col0 = 128 * j
                    pscr = aps.tile([128, 512], f32, tag="bank", bufs=3,
                                    name=f"pscrE{h}_{j}")
                    nc.tensor.matmul(pscr[:, col0:512],
                                     kT[:, 128 * j:128 * (j + 1)],
                                     qT[:, col0:512], start=True, stop=True)
                    pt = wp.tile([128, GW], bf16, tag="pt", bufs=48,
                                 name=f"ptE{h}_{j}")
                    nc.scalar.activation(pt[:, col0:512], pscr[:, col0:512],
                                         Exp, scale=0.125)
                    pts.append((pt, col0))
                return pts

            def emit_scores_late(h, pts):
                """g=0 pass A remainder: cols [512:1024] for k-chunks 0..3,
                full rows for k-chunks 4..7."""
                qm, qo = qv[h]
                km, ko = kv[h]
                qT = qkT[qm][qo:qo + 64, :]
                kT = qkT[km][ko:ko + 64, :]
                for j in range(GT):
                    col0 = 128 * j
                    lo = max(col0, 512)
                    pscr = aps.tile([128, GW], f32, tag="pscr", bufs=2,
                                    name=f"pscrL{h}_{j}")
                    for s0 in range(lo - lo % 512, GW, 512):
                        a0 = max(s0, lo)
                        nc.tensor.matmul(
                            pscr[:, a0:s0 + 512],
                            kT[:, 128 * j:128 * (j + 1)],
                            qT[:, a0:s0 + 512], start=True, stop=True)
                    if j < 4:
                        pt, _ = pts[j]
                    else:
                        pt = wp.tile([128, GW], bf16, tag="pt", bufs=48,
                                     name=f"ptL{h}_{j}")
                        pts.append((pt, col0))
                    nc.scalar.activation(pt[:, lo:], pscr[:, lo:],
                                         Exp, scale=0.125)
                return pts

            def _pv_finish(g, h, i, acc):
                rc = wp.tile([128, 1], f32, tag="rc", bufs=4,
                             name=f"rc{g}_{h}_{i}")
                nc.vector.reciprocal(rc[:], acc[:, 64:65])
                if h < 2:
                    dst = attn01[i][:, 64 * h:64 * (h + 1)]
                else:
                    dst = attn2[i][:]
                nc.vector.tensor_scalar_mul(dst, acc[:, 0:64], rc[:])

            def emit_pv_tile(g, h, i, pts):
                """P^T@V for q-tile i of group g: acc[q, 0:64]=o, [64]=denom."""
                jmax = GT * g + i
                pt_d, _ = pts[jmax]
                dcol = 128 * i
                nc.vector.tensor_tensor(pt_d[:, dcol:dcol + 128],
                                        pt_d[:, dcol:dcol + 128], tri[:],
                                        mult)
                acc = aps.tile([128, 512], f32, tag="bank", bufs=3,
                               name=f"acc{g}_{h}_{i}")
                for j in range(jmax + 1):
                    ptj, _ = pts[j]
                    nc.tensor.matmul(acc[:, 0:65],
                                     ptj[:, 128 * i:128 * (i + 1)],
                                     v_sb[:, j, h, :],
                                     start=(j == 0), stop=(j == jmax),
                                     skip_group_check=True)
                _pv_finish(g, h, i, acc)

            def emit_t01(g, i):
                ptr = aps.tile([128, 128], bf16, tag="ptr", bufs=1,
                               name=f"ptr01_{g}_{i}")
                nc.tensor.transpose(ptr[:], attn01[i][:], ident[:])
                csl = slice(GW * g + 128 * i, GW * g + 128 * (i + 1))
                nc.vector.tensor_copy(attnTa[:, csl], ptr[:])

            def emit_t2(g, i):
                ptr = aps.tile([128, 128], bf16, tag="ptr", bufs=1,
                               name=f"ptr2_{g}_{i}")
                nc.tensor.transpose(ptr[0:64, :], attn2[i][:], ident[:])
                csl = slice(GW * g + 128 * i, GW * g + 128 * (i + 1))
                nc.vector.tensor_copy(attnTb[:, csl], ptr[0:64, :])

            def emit_outproj(g, i):
                t = GT * g + i
                tsl = slice(128 * t, 128 * (t + 1))
                for c0, cn in ((0, 512), (512, 256)):
                    pout = aps.tile([128, 512], f32, tag="bank", bufs=3,
                                    name=f"pout{t}_{c0}")
                    nc.tensor.matmul(pout[:, 0:cn], attnTa[:, tsl],
                                     woA[:, c0:c0 + cn], start=True,
                                     stop=False)
                    nc.tensor.matmul(pout[:, 0:cn], attnTb[:, tsl],
                                     woB[:, c0:c0 + cn], start=False,
                                     stop=True)
                    osb = wp.tile([128, cn], bf16, tag=f"osb{c0}", bufs=3,
                                  name=f"osb{t}_{c0}")
                    # Pool has slack everywhere; keeps the epilogue off the
                    # DVE chain and the ACT exp stream
                    nc.gpsimd.tensor_copy(osb[:], pout[:, 0:cn])
                    # ACT is idle in the tail — split the final DMA issues
                    if g == 1 and i >= 3 and c0 == 0:
                        nc.scalar.dma_start(out_d[tsl, c0:c0 + cn], osb[:])
                    else:
                        dmaq[0].dma_start(out_d[tsl, c0:c0 + cn], osb[:])

            def emit_pv_block(g, h, pts, with_epilogue):
                """B pass; h==2 interleaves T01/T2/outproj epilogue."""
                for i in range(GT):
                    emit_pv_tile(g, h, i, pts)
                    if with_epilogue:
                        emit_t01(g, i)
                        if i >= 1:
                            emit_t2(g, i - 1)
                        if i >= 2:
                            emit_outproj(g, i - 2)
                if with_epilogue:
                    emit_t2(g, GT - 1)
                    emit_outproj(g, GT - 2)
                    emit_outproj(g, GT - 1)

            # ================= emission schedule =================
            # Software-pipelined so the ACT exp stream (the second
            # bottleneck) starts early and never waits long on scores.
            emit_qk_chunks([(0, 0), (1, 0)])
            pts00 = emit_scores_early(0)
            pts01 = emit_scores_early(1)
            emit_qk_chunks([(0, 1), (1, 1)])
            pts00 = emit_scores_late(0, pts00)
            pts01 = emit_scores_late(1, pts01)
            emit_qk_chunks([(2, 0), (2, 1)])
            pts02 = emit_scores(0, 2)
            emit_qk_chunks([(m, n) for n in (2, 3) for m in (0, 1, 2)])
            # g1 score passes are exp-paced; fill their PE waits with the
            # independent V-proj, g0 PV blocks, and the g0 epilogue.
            def mk(f, *a):
                return lambda: f(*a)

            fil0 = [mk(emit_v, t) for t in range(8)]
            fil0 += [mk(emit_pv_tile, 0, 0, i, pts00) for i in range(GT)]
            pts10 = emit_scores(1, 0, fil0)

            fil1 = [mk(emit_pv_tile, 0, 1, i, pts01) for i in range(GT)]
            fil1 += [mk(emit_v, t) for t in range(8, 16)]
            pts11 = emit_scores(1, 1, fil1)

            def mk_epi0(i):
                def f():
                    emit_pv_tile(0, 2, i, pts02)
                    emit_t01(0, i)
                    if i >= 1:
                        emit_t2(0, i - 1)
                    if i >= 2:
                        emit_outproj(0, i - 2)
                    if i == GT - 1:
                        emit_t2(0, GT - 1)
                        emit_outproj(0, GT - 2)
                        emit_outproj(0, GT - 1)
                return f

            fil2 = [mk_epi0(i) for i in range(GT)]
            fil2 += [mk(emit_pv_tile, 1, 0, i, pts10) for i in range(GT)]
            pts12 = emit_scores(1, 2, fil2)

            emit_pv_block(1, 1, pts11, False)
            emit_pv_block(1, 2, pts12, True)

    nc.compile()
    return nc


def _host_inputs(x, w_qkv, w_out):
    """Build the 8 per-core input maps (bf16)."""
    import ml_dtypes
    bf = ml_dtypes.bfloat16

    inv_freq = 1.0 / (ROPE_BASE ** (np.arange(0, D, 2, dtype=np.float32) / D))
    t = np.arange(T, dtype=np.float32)
    freqs = t[:, None] * inv_freq[None, :]          # [T, D/2]
    emb = np.concatenate([freqs, freqs], axis=-1)   # [T, D]
    cosT = np.ascontiguousarray(np.cos(emb).T)      # [D, T]
    sinT = np.ascontiguousarray(np.sin(emb).T)
    cosT2 = np.concatenate([cosT, cosT], axis=0).astype(bf)   # [128, T]
    sinT2 = np.concatenate([sinT, sinT], axis=0).astype(bf)

    # rotate_half permutation as matmul lhsT: rot = R @ q, lhsT = R.T
    R = np.zeros((D, D), np.float32)
    R[0:32, 32:64] = -np.eye(32)
    R[32:64, 0:32] = np.eye(32)
    R2 = np.zeros((128, 128), np.float32)
    R2[0:64, 0:64] = R
    R2[64:128, 64:128] = R
    rT = np.ascontiguousarray(R2.T).astype(bf)

    tri = np.zeros((128, 128), np.float32)
    for kr in range(128):
        tri[kr, kr:] = 1.0
    tri = tri.astype(bf)
    ident = np.eye(128, dtype=np.float32).astype(bf)

    wq = w_qkv[0:C]
    wk = w_qkv[C:2 * C]
    wv = w_qkv[2 * C:3 * C]

    maps = []
    for core in range(NG):
        b, hg = core // 4, core % 4
        hs = slice(HPG * D * hg, HPG * D * (hg + 1))   # 192 rows of this group
        h2 = HPG * D * hg + 2 * D
        q01 = wq[hs][0:128]                             # [128, C]
        k01 = wk[hs][0:128]
        q2 = wq[h2:h2 + D]
        k2 = wk[h2:h2 + D]
        v3 = wv[hs]                                     # [192, C]
        wA = np.zeros((C, 576), np.float32)
        wA[:, 0:128] = q01.T
        wA[:, 128:256] = k01.T
        wA[:, 256:320] = q2.T
        wA[:, 320:384] = k2.T
        wA[:, 384:576] = v3.T
        wo_h = [w_out[:, HPG * D * hg + D * h: HPG * D * hg + D * (h + 1)].T
                for h in range(HPG)]                    # 3 x [64, C]
        woA = np.concatenate([wo_h[0], wo_h[1]], axis=0)  # [128, C]
        woB = wo_h[2]                                     # [64, C]
        maps.append({
            "xT": np.ascontiguousarray(x[b].T).astype(bf),
            "wA": np.ascontiguousarray(wA).astype(bf),
            "woA": np.ascontiguousarray(woA).astype(bf),
            "woB": np.ascontiguousarray(woB).astype(bf),
            "cosT": cosT2, "sinT": sinT2,
            "rT": rT, "tri": tri, "ident": ident,
        })
    return maps


def kernel(x, w_qkv, w_out):
    from concourse.bass_utils import run_bass_kernel_spmd

    if "nc" not in _CACHE:
        _CACHE["nc"] = _build_nc()
    nc = _CACHE["nc"]

    maps = _host_inputs(np.asarray(x, np.float32),
                        np.asarray(w_qkv, np.float32),
                        np.asarray(w_out, np.float32))
    res = run_bass_kernel_spmd(nc, maps, core_ids=list(range(NG))).results
    parts = np.stack([np.asarray(r["out"], dtype=np.float32)
                      for r in res])                    # [8, T, C]
    out = np.zeros((B, T, C), np.float32)
    for b in range(B):
        out[b] = parts[4 * b:4 * (b + 1)].sum(axis=0)
    return out
